# revision 35
# baseline (speedup 1.0000x reference)
"""MoE LoRA layer kernel for 8 Trainium2 NeuronCores.

Strategy (data-parallel over B):
  - B=16 batches sharded 2 per core. Gate / LoRA params replicated.
  - Host side only re-lays-out data (transpose/reshape) for sharding; all
    module arithmetic over tokens runs on device:
      * pooled mean + router logits (fp32 matmuls)
      * top-2 + softmax -> per-(expert,rank) scale vector s  (on device)
      * stage 1: u^T[er, n] = lora_a_all @ tokens^T      (fp32r matmuls)
      * stage 2: upd[n, c]  = (u*s)^T @ lora_b_all^T     (fp32r matmuls)
  - Top-2 selection is folded into a dense ER=128 contraction: u columns are
    scaled by s[er] = expert_weights[e] * scaling[e] (zero for non-selected
    experts), so no data-dependent control flow is needed on device.
  - Device returns per-core weighted_update shard + router_logits rows;
    the [16,8]-sized routing outputs (top_idx/expert_weights/importance/load)
    are reassembled on host from the device-computed logits.

Pipelining: loads are ordered (lora_a, gate, batch-0 tokens, scaling,
lora_b, batch-1 tokens) so PE can start stage-1 of batch 0 while batch 1
is still loading; both batches' stage-1 + routing run before the stage-2
flood so the routing latency chain stays off the critical path.
"""

import sys

sys.path.insert(0, "/opt/trn_rl_repo")

import numpy as np

import concourse.bass as bass
import concourse.mybir as mybir
from concourse.tile import TileContext
from concourse.bass_utils import run_bass_kernel_spmd

B, N, D, E, R, C3, TOP_K = 16, 577, 1024, 8, 16, 3072, 2
NCORES = 8
BPC = B // NCORES          # batches per core
ER = E * R                 # 128
KT = D // 128              # 8 k-tiles over the D contraction
NT = BPC * N               # tokens per core
F32 = mybir.dt.float32
F32R = mybir.dt.float32r
AF = mybir.ActivationFunctionType

# Stage-1 free-dim chunking. fp32r matmuls need even moving/dst free counts,
# so tokens tiles are padded from N=577 to NP=578 columns (pad column zeroed);
# chunks stay >=256 so fp32r runs at full rate.
NP = 578
S1_CHUNKS = [(0, 290), (290, 288)]
CCH = 6                    # stage-2 c chunks of 512


def _split_waits(nc):
    """This container's walrus rejects >1 sync-wait per instruction (and any
    wait on a Matmult, whose 4-byte LDWEIGHTS half has no wait slot). Hoist
    excess waits onto standalone EventSemaphore NOPs just before, preserving
    per-engine program order."""
    n = 0
    for fn in nc.m.functions:
        for bb in fn.blocks:
            out = []
            changed = False
            for inst in bb.instructions:
                si = inst.sync_info
                maxw = 0 if type(inst).__name__ == "InstMatmult" else 1
                if si is not None and len(si.on_wait) > maxw:
                    waits = list(si.on_wait)
                    keep = waits[len(waits) - maxw:] if maxw else []
                    for w in waits[:len(waits) - maxw]:
                        ev = mybir.InstEventSemaphore(
                            name=f"{inst.name}-hw{n}", ins=[], outs=[],
                            sync_info=mybir.SyncInfo(on_wait=[w], on_update=[]))
                        ev.engine = inst.engine
                        out.append(ev)
                        n += 1
                    inst.sync_info = mybir.SyncInfo(
                        on_wait=keep, on_update=list(si.on_update))
                    changed = True
                out.append(inst)
            if changed:
                bb.instructions = out
    return n


def _build_nc():
    nc = bass.Bass()
    # tok_h is partition-major and pre-padded on the host so each load is a
    # single DMA with multi-KB descriptors (full DMA-engine efficiency)
    tok_h = nc.declare_dram_parameter("tok_h", [BPC, 128, KT, NP], F32R,
                                      isOutput=False)
    gate_h = nc.declare_dram_parameter("gate_h", [128, KT, E], F32,
                                       isOutput=False)
    lora_ah = nc.declare_dram_parameter("lora_ah", [128, KT, 128], F32R,
                                        isOutput=False)
    lora_bt = nc.declare_dram_parameter("lora_bt", [ER, C3], F32R, isOutput=False)
    srep = nc.declare_dram_parameter("srep", [ER, 1], F32, isOutput=False)
    upd = nc.declare_dram_parameter("upd", [NT, C3], F32, isOutput=True)
    logits = nc.declare_dram_parameter("logits", [BPC, E], F32, isOutput=True)

    with TileContext(nc) as tc:
        with tc.tile_pool(name="const", bufs=1) as cpool, \
             tc.tile_pool(name="tok", bufs=1) as tokpool, \
             tc.tile_pool(name="usb", bufs=1) as upool, \
             tc.tile_pool(name="osb", bufs=4) as opool, \
             tc.tile_pool(name="small", bufs=4) as spool, \
             tc.tile_pool(name="ps_small", bufs=1, space="PSUM") as ps_small, \
             tc.tile_pool(name="ps_big", bufs=7, space="PSUM") as ps_big:

            # ---- loads: few big-descriptor DMAs, ordered for pipelining.
            # Token batches come in halves so stage-1 can start early. ----
            la = cpool.tile([128, KT, 128], F32R)
            nc.sync.dma_start(la[:, :, :], lora_ah[:, :, :])
            tokb = {}
            for b in range(BPC):
                tokb[b] = tokpool.tile([128, KT, NP], F32R, name=f"tok{b}")
            for q in range(4):
                nc.sync.dma_start(tokb[0][:, 2 * q:2 * q + 2, :],
                                  tok_h[0, :, 2 * q:2 * q + 2, :])
            gw = cpool.tile([128, KT, E], F32)
            nc.sync.dma_start(gw[:, :, :], gate_h[:, :, :])
            srt = cpool.tile([128, 1], F32)
            nc.sync.dma_start(srt[:, :], srep[:, :])
            bw = cpool.tile([128, C3], F32R)
            nc.sync.dma_start(bw[:, :], lora_bt[:, :])
            for q in range(4):
                nc.sync.dma_start(tokb[1][:, 2 * q:2 * q + 2, :],
                                  tok_h[1, :, 2 * q:2 * q + 2, :])
            tk = {(b, k): tokb[b][:, k, :] for b in range(BPC)
                  for k in range(KT)}
            onet = cpool.tile([1, 1], F32)
            nc.vector.memset(onet[:, :], 1.0)
            # touch Exp once so the ~1.3us ACT_TABLE_LOAD happens during the
            # load phase instead of on the routing critical path
            expwarm = cpool.tile([1, 1], F32)
            nc.scalar.activation(expwarm[:, :], onet[:, :], AF.Exp)

            # ---- emission helpers ----
            u = {}
            s_col = {}

            def stage1_mms(b):
                """Stage-1 matmul groups for batch b, k-major so PE paces
                with token-tile arrival; returns the psum chunk tiles."""
                ups = [ps_big.tile([128, 512], F32, name=f"up{b}_{c0}",
                                   tag="ps")
                       for (c0, cw) in S1_CHUNKS]
                for k in range(KT):
                    for (c0, cw), up in zip(S1_CHUNKS, ups):
                        nc.tensor.matmul(up[:, :cw], la[:, k, :],
                                         tk[(b, k)][:, c0:c0 + cw],
                                         start=(k == 0), stop=(k == KT - 1))
                return ups

            def routing_and_evict(b, ups):
                # pooled sums: free-dim reduce split across DVE (reduce) and
                # ACT (Copy with accum_out side-sum into a scratch), so the
                # serial latency before the router halves
                pooled = spool.tile([128, KT], F32, name=f"pooled{b}", bufs=1)
                for k in range(KT):
                    if k % 2 == 0:
                        scr = spool.tile([128, N], F32, name="pscr",
                                         tag="pscr", bufs=2)
                        nc.scalar.activation(scr[:, :],
                                             tk[(b, k)][:, :N].bitcast(F32),
                                             AF.Copy,
                                             accum_out=pooled[:, k:k + 1])
                    else:
                        nc.vector.reduce_sum(pooled[:, k:k + 1],
                                             tk[(b, k)][:, :N].bitcast(F32),
                                             axis=mybir.AxisListType.X)

                # router logits [1, E] (fp32, exact)
                lg_ps = ps_small.tile([1, E], F32, name=f"lg{b}", tag="pss")
                for k in range(KT):
                    nc.tensor.matmul(lg_ps[:, :], pooled[:, k:k + 1],
                                     gw[:, k, :], start=(k == 0),
                                     stop=(k == KT - 1))
                r_sb = spool.tile([1, E], F32, name=f"r{b}", bufs=1)
                nc.scalar.activation(r_sb[:, :], lg_ps[:, :], AF.Copy,
                                     scale=1.0 / N)
                nc.sync.dma_start(logits[b:b + 1, :], r_sb[:, :])
                # logits row with each entry replicated R times: the top-2
                # chain below then directly yields the [er]-indexed weights
                r128 = spool.tile([1, ER], F32, name=f"r128{b}", bufs=1)
                nc.vector.tensor_scalar_mul(
                    r128[:, :].rearrange("p (e r) -> p e r", r=R),
                    lg_ps[:, :, None].broadcast_to((1, E, R)), 1.0 / N)

                # top-2 softmax on the replicated row -> ew row [1, ER]
                m1 = spool.tile([1, 1], F32, name=f"m1{b}", bufs=1)
                nc.vector.reduce_max(m1[:, :], r128[:, :],
                                     axis=mybir.AxisListType.X)
                eq1 = spool.tile([1, ER], F32, name=f"eq1{b}", bufs=1)
                nc.vector.tensor_scalar(eq1[:, :], r128[:, :], m1[:, :], None,
                                        op0=mybir.AluOpType.is_equal)
                msk = spool.tile([1, ER], F32, name=f"msk{b}", bufs=1)
                nc.vector.scalar_tensor_tensor(
                    msk[:, :], eq1[:, :], -1e30, r128[:, :],
                    op0=mybir.AluOpType.mult, op1=mybir.AluOpType.add)
                m2 = spool.tile([1, 1], F32, name=f"m2{b}", bufs=1)
                nc.vector.reduce_max(m2[:, :], msk[:, :],
                                     axis=mybir.AxisListType.X)
                eq2 = spool.tile([1, ER], F32, name=f"eq2{b}", bufs=1)
                nc.vector.tensor_scalar(eq2[:, :], msk[:, :], m2[:, :], None,
                                        op0=mybir.AluOpType.is_equal)
                nm1 = spool.tile([1, 1], F32, name=f"nm1{b}", bufs=1)
                nc.vector.tensor_scalar_mul(nm1[:, :], m1[:, :], -1.0)
                dex = spool.tile([1, 1], F32, name=f"dex{b}", bufs=1)
                nc.scalar.activation(dex[:, :], m2[:, :], AF.Exp,
                                     bias=nm1[:, :], scale=1.0)
                t1 = spool.tile([1, 1], F32, name=f"t1{b}", bufs=1)
                nc.vector.tensor_scalar_add(t1[:, :], dex[:, :], 1.0)
                w1 = spool.tile([1, 1], F32, name=f"w1{b}", bufs=1)
                nc.vector.reciprocal(w1[:, :], t1[:, :])
                w2 = spool.tile([1, 1], F32, name=f"w2{b}", bufs=1)
                nc.vector.tensor_mul(w2[:, :], dex[:, :], w1[:, :])
                p1 = spool.tile([1, ER], F32, name=f"p1{b}", bufs=1)
                nc.vector.tensor_scalar(p1[:, :], eq1[:, :], w1[:, :], None,
                                        op0=mybir.AluOpType.mult)
                p2 = spool.tile([1, ER], F32, name=f"p2{b}", bufs=1)
                nc.vector.tensor_scalar(p2[:, :], eq2[:, :], w2[:, :], None,
                                        op0=mybir.AluOpType.mult)
                ew = spool.tile([1, ER], F32, name=f"ew{b}", bufs=1)
                nc.vector.tensor_add(ew[:, :], p1[:, :], p2[:, :])

                # transpose the ew row to a per-partition column via matmul
                s_ps = ps_small.tile([128, 1], F32, name=f"sp{b}", tag="pss")
                nc.tensor.matmul(s_ps[:, :], ew[:, :], onet[:, :],
                                 start=True, stop=True)
                s0 = spool.tile([128, 1], F32, name=f"s0{b}", bufs=1)
                nc.scalar.copy(s0[:, :], s_ps[:, :])
                sc = spool.tile([128, 1], F32, name=f"scol{b}", bufs=1)
                nc.vector.tensor_mul(sc[:, :], s0[:, :], srt[:, :])
                s_col[b] = sc

                # scaled eviction of u (folds in gate weight * lora scaling)
                ub = upool.tile([128, NP], F32R, name=f"u{b}")
                for (c0, cw), up in zip(S1_CHUNKS, ups):
                    nc.scalar.activation(ub[:, c0:c0 + cw], up[:, :cw],
                                         AF.Copy, scale=sc[:, :])
                u[b] = ub

            # stage-2 unit emitter: one (nt, cc) matmul + eviction (+ store
            # after the last chunk of an n-tile)
            osb_cur = {}

            def stage2_unit(b, nt, cc):
                n0 = nt * 128
                nsz = min(128, N - n0)
                if cc == 0:
                    osb_cur[b] = opool.tile([128, C3], F32, name="osb",
                                            tag="osb")
                osb = osb_cur[b]
                op_ps = ps_big.tile([128, 512], F32,
                                    name=f"op{b}_{nt}_{cc}", tag="ps")
                nc.tensor.matmul(op_ps[:nsz, :], u[b][:, n0:n0 + nsz],
                                 bw[:, cc * 512:(cc + 1) * 512],
                                 start=True, stop=True)
                if cc % 2 == 0:
                    nc.scalar.copy(osb[:nsz, cc * 512:(cc + 1) * 512],
                                   op_ps[:nsz, :])
                else:
                    nc.vector.tensor_copy(
                        osb[:nsz, cc * 512:(cc + 1) * 512], op_ps[:nsz, :])
                if cc == CCH - 1:
                    nc.sync.dma_start(upd[b * N + n0:b * N + n0 + nsz, :],
                                      osb[:nsz, :])

            NTILES = (N + 127) // 128
            s2_units = [(nt, cc) for nt in range(NTILES) for cc in range(CCH)]

            # ---- emission schedule ----
            # batch 0: stage 1, routing, eviction
            ups0 = stage1_mms(0)
            routing_and_evict(0, ups0)
            # batch 0 stage 2 interleaved with batch 1 stage 1 (two k-slices
            # after each n-tile), so PE stays busy through batch-1 token
            # arrival and the output stream starts as early as possible
            ups1 = [ps_big.tile([128, 512], F32, name=f"up1_{c0}", tag="ps")
                    for (c0, cw) in S1_CHUNKS]
            for nt in range(KT // 2):
                for cc in range(CCH):
                    stage2_unit(0, nt, cc)
                for k in (2 * nt, 2 * nt + 1):
                    for (c0, cw), up in zip(S1_CHUNKS, ups1):
                        nc.tensor.matmul(up[:, :cw], la[:, k, :],
                                         tk[(1, k)][:, c0:c0 + cw],
                                         start=(k == 0), stop=(k == KT - 1))
            # batch-1 routing before the last batch-0 n-tiles, so its serial
            # chain overlaps stage-2 work instead of stalling PE afterwards
            routing_and_evict(1, ups1)
            for nt in range(KT // 2, NTILES):
                for cc in range(CCH):
                    stage2_unit(0, nt, cc)
            # batch 1 stage 2
            for nt in range(NTILES):
                for cc in range(CCH):
                    stage2_unit(1, nt, cc)

    _split_waits(nc)
    return nc


_NC_CACHE = {}


def _get_nc():
    if "nc" not in _NC_CACHE:
        _NC_CACHE["nc"] = _build_nc()
    return _NC_CACHE["nc"]


def make_in_maps(tokens, gate_w, lora_a, lora_b, scaling):
    # replicated params, kernel-friendly layouts (pure relayout, no math)
    # lora_ah[p, k, m] = lora_a[(m//R), (m%R), k*128+p]
    lora_ah = np.ascontiguousarray(
        lora_a.reshape(ER, KT, 128).transpose(2, 1, 0))  # [128, KT, ER]
    lora_bt = np.ascontiguousarray(
        lora_b.transpose(0, 2, 1).reshape(ER, C3))       # [ER, C3]
    # gate_h[p, k, e] = gate_w[k*128+p, e]
    gate_h = np.ascontiguousarray(
        gate_w.reshape(KT, 128, E).transpose(1, 0, 2))   # [128, KT, E]
    srep = np.ascontiguousarray(
        np.repeat(scaling, R).reshape(ER, 1))            # [ER, 1]
    in_maps = []
    for c in range(NCORES):
        shard = tokens[c * BPC:(c + 1) * BPC]            # [BPC, N, D]
        # tok_h[b, p, k, n] = tokens[b, n, k*128+p], zero-padded to NP cols
        tok_h = np.zeros((BPC, 128, KT, NP), dtype=np.float32)
        tok_h[:, :, :, :N] = shard.transpose(0, 2, 1).reshape(
            BPC, KT, 128, N).transpose(0, 2, 1, 3)
        in_maps.append({
            "tok_h": tok_h, "gate_h": gate_h, "lora_ah": lora_ah,
            "lora_bt": lora_bt, "srep": srep,
        })
    return in_maps


def kernel(tokens, gate_w, lora_a, lora_b, scaling):
    tokens = np.asarray(tokens, dtype=np.float32)
    gate_w = np.asarray(gate_w, dtype=np.float32)
    lora_a = np.asarray(lora_a, dtype=np.float32)
    lora_b = np.asarray(lora_b, dtype=np.float32)
    scaling = np.asarray(scaling, dtype=np.float32)

    in_maps = make_in_maps(tokens, gate_w, lora_a, lora_b, scaling)
    res = run_bass_kernel_spmd(_get_nc(), in_maps, core_ids=list(range(NCORES)))

    weighted_update = np.empty((B, N, C3), dtype=np.float32)
    router_logits = np.empty((B, E), dtype=np.float32)
    for c in range(NCORES):
        weighted_update[c * BPC:(c + 1) * BPC] = \
            res.results[c]["upd"].reshape(BPC, N, C3)
        router_logits[c * BPC:(c + 1) * BPC] = res.results[c]["logits"]

    # [16,8]-sized routing outputs, derived from device-computed logits
    top_idx = np.argsort(-router_logits, axis=-1, kind="stable")[:, :TOP_K]
    top_idx = np.ascontiguousarray(top_idx).astype(np.int32)
    top_vals = np.take_along_axis(router_logits, top_idx, axis=1)
    ex = np.exp(top_vals - top_vals.max(axis=1, keepdims=True))
    top_w = (ex / ex.sum(axis=1, keepdims=True)).astype(np.float32)
    expert_weights = np.zeros((B, E), dtype=np.float32)
    np.put_along_axis(expert_weights, top_idx, top_w, axis=1)
    importance = expert_weights.sum(axis=0)
    load = np.zeros((E,), dtype=np.float32)
    np.add.at(load, top_idx.reshape(-1), 1.0)

    return (weighted_update, router_logits, top_idx, expert_weights,
            importance, load)


# revision 36
# speedup vs baseline: 1.1372x; 1.1372x over previous
"""MoE LoRA layer kernel for 8 Trainium2 NeuronCores.

Strategy (data-parallel over B):
  - B=16 batches sharded 2 per core. Gate / LoRA params replicated.
  - Host side only re-lays-out data (transpose/reshape) for sharding; all
    module arithmetic over tokens runs on device:
      * pooled mean + router logits (fp32 matmuls)
      * top-2 + softmax -> per-(expert,rank) scale vector s  (on device)
      * stage 1: u^T[er, n] = lora_a_all @ tokens^T      (fp32r matmuls)
      * stage 2: upd[n, c]  = (u*s)^T @ lora_b_all^T     (fp32r matmuls)
  - Top-2 selection is folded into a dense ER=128 contraction: u columns are
    scaled by s[er] = expert_weights[e] * scaling[e] (zero for non-selected
    experts), so no data-dependent control flow is needed on device.
  - Device returns per-core weighted_update shard + router_logits rows;
    the [16,8]-sized routing outputs (top_idx/expert_weights/importance/load)
    are reassembled on host from the device-computed logits.

Pipelining: loads are ordered (lora_a, gate, batch-0 tokens, scaling,
lora_b, batch-1 tokens) so PE can start stage-1 of batch 0 while batch 1
is still loading; both batches' stage-1 + routing run before the stage-2
flood so the routing latency chain stays off the critical path.
"""

import sys

sys.path.insert(0, "/opt/trn_rl_repo")

import numpy as np

import concourse.bass as bass
import concourse.mybir as mybir
from concourse.tile import TileContext
from concourse.bass_utils import run_bass_kernel_spmd

B, N, D, E, R, C3, TOP_K = 16, 577, 1024, 8, 16, 3072, 2
NCORES = 8
BPC = B // NCORES          # batches per core
ER = E * R                 # 128
KT = D // 128              # 8 k-tiles over the D contraction
NT = BPC * N               # tokens per core
F32 = mybir.dt.float32
F32R = mybir.dt.float32r
AF = mybir.ActivationFunctionType

# Stage-1 free-dim chunking. fp32r matmuls need even moving/dst free counts,
# so tokens tiles are padded from N=577 to NP=578 columns (pad column zeroed);
# chunks stay >=256 so fp32r runs at full rate.
NP = 578
S1_CHUNKS = [(0, 290), (290, 288)]
CCH = 6                    # stage-2 c chunks of 512


def _split_waits(nc):
    """This container's walrus rejects >1 sync-wait per instruction (and any
    wait on a Matmult, whose 4-byte LDWEIGHTS half has no wait slot). Hoist
    excess waits onto standalone EventSemaphore NOPs just before, preserving
    per-engine program order."""
    n = 0
    for fn in nc.m.functions:
        for bb in fn.blocks:
            out = []
            changed = False
            for inst in bb.instructions:
                si = inst.sync_info
                maxw = 0 if type(inst).__name__ == "InstMatmult" else 1
                if si is not None and len(si.on_wait) > maxw:
                    waits = list(si.on_wait)
                    keep = waits[len(waits) - maxw:] if maxw else []
                    for w in waits[:len(waits) - maxw]:
                        ev = mybir.InstEventSemaphore(
                            name=f"{inst.name}-hw{n}", ins=[], outs=[],
                            sync_info=mybir.SyncInfo(on_wait=[w], on_update=[]))
                        ev.engine = inst.engine
                        out.append(ev)
                        n += 1
                    inst.sync_info = mybir.SyncInfo(
                        on_wait=keep, on_update=list(si.on_update))
                    changed = True
                out.append(inst)
            if changed:
                bb.instructions = out
    return n


def _build_nc():
    nc = bass.Bass()
    # tok_h is partition-major and pre-padded on the host so each load is a
    # single DMA with multi-KB descriptors (full DMA-engine efficiency)
    tok_h = nc.declare_dram_parameter("tok_h", [BPC, 128, KT, NP], F32R,
                                      isOutput=False)
    gate_h = nc.declare_dram_parameter("gate_h", [128, KT, E], F32,
                                       isOutput=False)
    lora_ah = nc.declare_dram_parameter("lora_ah", [128, KT, 128], F32R,
                                        isOutput=False)
    lora_bt = nc.declare_dram_parameter("lora_bt", [ER, C3], F32R, isOutput=False)
    srep = nc.declare_dram_parameter("srep", [ER, 1], F32, isOutput=False)
    upd = nc.declare_dram_parameter("upd", [NT, C3], F32, isOutput=True)
    logits = nc.declare_dram_parameter("logits", [BPC, E], F32, isOutput=True)

    with TileContext(nc) as tc:
        with tc.tile_pool(name="const", bufs=1) as cpool, \
             tc.tile_pool(name="tok", bufs=1) as tokpool, \
             tc.tile_pool(name="usb", bufs=1) as upool, \
             tc.tile_pool(name="osb", bufs=4) as opool, \
             tc.tile_pool(name="small", bufs=4) as spool, \
             tc.tile_pool(name="ps_small", bufs=1, space="PSUM") as ps_small, \
             tc.tile_pool(name="ps_big", bufs=7, space="PSUM") as ps_big:

            # ---- loads: few big-descriptor DMAs, ordered for pipelining.
            # Token batches come in halves so stage-1 can start early. ----
            la = cpool.tile([128, KT, 128], F32R)
            nc.sync.dma_start(la[:, :, :], lora_ah[:, :, :])
            tokb = {}
            for b in range(BPC):
                tokb[b] = tokpool.tile([128, KT, NP], F32R, name=f"tok{b}")
            for q in range(4):
                nc.sync.dma_start(tokb[0][:, 2 * q:2 * q + 2, :],
                                  tok_h[0, :, 2 * q:2 * q + 2, :])
            gw = cpool.tile([128, KT, E], F32)
            nc.sync.dma_start(gw[:, :, :], gate_h[:, :, :])
            srt = cpool.tile([128, 1], F32)
            nc.sync.dma_start(srt[:, :], srep[:, :])
            bw = cpool.tile([128, C3], F32R)
            nc.sync.dma_start(bw[:, :], lora_bt[:, :])
            for q in range(4):
                nc.sync.dma_start(tokb[1][:, 2 * q:2 * q + 2, :],
                                  tok_h[1, :, 2 * q:2 * q + 2, :])
            tk = {(b, k): tokb[b][:, k, :] for b in range(BPC)
                  for k in range(KT)}
            onet = cpool.tile([1, 1], F32)
            nc.vector.memset(onet[:, :], 1.0)
            # touch Exp once so the ~1.3us ACT_TABLE_LOAD happens during the
            # load phase instead of on the routing critical path
            expwarm = cpool.tile([1, 1], F32)
            nc.scalar.activation(expwarm[:, :], onet[:, :], AF.Exp)

            # ---- emission helpers ----
            u = {}
            s_col = {}

            def stage1_mms(b):
                """Stage-1 matmul groups for batch b, k-major so PE paces
                with token-tile arrival; returns the psum chunk tiles."""
                ups = [ps_big.tile([128, 512], F32, name=f"up{b}_{c0}",
                                   tag="ps")
                       for (c0, cw) in S1_CHUNKS]
                for k in range(KT):
                    for (c0, cw), up in zip(S1_CHUNKS, ups):
                        nc.tensor.matmul(up[:, :cw], la[:, k, :],
                                         tk[(b, k)][:, c0:c0 + cw],
                                         start=(k == 0), stop=(k == KT - 1))
                return ups

            def routing_and_evict(b, ups):
                # pooled sums: free-dim reduce split across DVE (reduce) and
                # ACT (Copy with accum_out side-sum into a scratch), so the
                # serial latency before the router halves
                pooled = spool.tile([128, KT], F32, name=f"pooled{b}", bufs=1)
                for k in range(KT):
                    if k % 2 == 0:
                        scr = spool.tile([128, N], F32, name="pscr",
                                         tag="pscr", bufs=2)
                        nc.scalar.activation(scr[:, :],
                                             tk[(b, k)][:, :N].bitcast(F32),
                                             AF.Copy,
                                             accum_out=pooled[:, k:k + 1])
                    else:
                        nc.vector.reduce_sum(pooled[:, k:k + 1],
                                             tk[(b, k)][:, :N].bitcast(F32),
                                             axis=mybir.AxisListType.X)

                # router logits [1, E] (fp32, exact)
                lg_ps = ps_small.tile([1, E], F32, name=f"lg{b}", tag="pss")
                for k in range(KT):
                    nc.tensor.matmul(lg_ps[:, :], pooled[:, k:k + 1],
                                     gw[:, k, :], start=(k == 0),
                                     stop=(k == KT - 1))
                r_sb = spool.tile([1, E], F32, name=f"r{b}", bufs=1)
                nc.scalar.activation(r_sb[:, :], lg_ps[:, :], AF.Copy,
                                     scale=1.0 / N)
                nc.sync.dma_start(logits[b:b + 1, :], r_sb[:, :])
                # logits row with each entry replicated R times: the top-2
                # chain below then directly yields the [er]-indexed weights
                r128 = spool.tile([1, ER], F32, name=f"r128{b}", bufs=1)
                nc.vector.tensor_scalar_mul(
                    r128[:, :].rearrange("p (e r) -> p e r", r=R),
                    lg_ps[:, :, None].broadcast_to((1, E, R)), 1.0 / N)

                # top-2 softmax on the replicated row -> ew row [1, ER]
                m1 = spool.tile([1, 1], F32, name=f"m1{b}", bufs=1)
                nc.vector.reduce_max(m1[:, :], r128[:, :],
                                     axis=mybir.AxisListType.X)
                eq1 = spool.tile([1, ER], F32, name=f"eq1{b}", bufs=1)
                nc.vector.tensor_scalar(eq1[:, :], r128[:, :], m1[:, :], None,
                                        op0=mybir.AluOpType.is_equal)
                msk = spool.tile([1, ER], F32, name=f"msk{b}", bufs=1)
                nc.vector.scalar_tensor_tensor(
                    msk[:, :], eq1[:, :], -1e30, r128[:, :],
                    op0=mybir.AluOpType.mult, op1=mybir.AluOpType.add)
                m2 = spool.tile([1, 1], F32, name=f"m2{b}", bufs=1)
                nc.vector.reduce_max(m2[:, :], msk[:, :],
                                     axis=mybir.AxisListType.X)
                eq2 = spool.tile([1, ER], F32, name=f"eq2{b}", bufs=1)
                nc.vector.tensor_scalar(eq2[:, :], msk[:, :], m2[:, :], None,
                                        op0=mybir.AluOpType.is_equal)
                nm1 = spool.tile([1, 1], F32, name=f"nm1{b}", bufs=1)
                nc.vector.tensor_scalar_mul(nm1[:, :], m1[:, :], -1.0)
                dex = spool.tile([1, 1], F32, name=f"dex{b}", bufs=1)
                nc.scalar.activation(dex[:, :], m2[:, :], AF.Exp,
                                     bias=nm1[:, :], scale=1.0)
                t1 = spool.tile([1, 1], F32, name=f"t1{b}", bufs=1)
                nc.vector.tensor_scalar_add(t1[:, :], dex[:, :], 1.0)
                w1 = spool.tile([1, 1], F32, name=f"w1{b}", bufs=1)
                nc.vector.reciprocal(w1[:, :], t1[:, :])
                w2 = spool.tile([1, 1], F32, name=f"w2{b}", bufs=1)
                nc.vector.tensor_mul(w2[:, :], dex[:, :], w1[:, :])
                p1 = spool.tile([1, ER], F32, name=f"p1{b}", bufs=1)
                nc.vector.tensor_scalar(p1[:, :], eq1[:, :], w1[:, :], None,
                                        op0=mybir.AluOpType.mult)
                p2 = spool.tile([1, ER], F32, name=f"p2{b}", bufs=1)
                nc.vector.tensor_scalar(p2[:, :], eq2[:, :], w2[:, :], None,
                                        op0=mybir.AluOpType.mult)
                ew = spool.tile([1, ER], F32, name=f"ew{b}", bufs=1)
                nc.vector.tensor_add(ew[:, :], p1[:, :], p2[:, :])

                # transpose the ew row to a per-partition column via matmul
                s_ps = ps_small.tile([128, 1], F32, name=f"sp{b}", tag="pss")
                nc.tensor.matmul(s_ps[:, :], ew[:, :], onet[:, :],
                                 start=True, stop=True)
                s0 = spool.tile([128, 1], F32, name=f"s0{b}", bufs=1)
                nc.scalar.copy(s0[:, :], s_ps[:, :])
                sc = spool.tile([128, 1], F32, name=f"scol{b}", bufs=1)
                nc.vector.tensor_mul(sc[:, :], s0[:, :], srt[:, :])
                s_col[b] = sc

                # scaled eviction of u (folds in gate weight * lora scaling)
                ub = upool.tile([128, NP], F32R, name=f"u{b}")
                for (c0, cw), up in zip(S1_CHUNKS, ups):
                    nc.scalar.activation(ub[:, c0:c0 + cw], up[:, :cw],
                                         AF.Copy, scale=sc[:, :])
                u[b] = ub

            # stage-2 unit emitter: one (nt, cc) matmul + eviction (+ store
            # after the last chunk of an n-tile)
            osb_cur = {}

            def stage2_unit(b, nt, cc):
                n0 = nt * 128
                nsz = min(128, N - n0)
                if cc == 0:
                    osb_cur[b] = opool.tile([128, C3], F32, name="osb",
                                            tag="osb")
                osb = osb_cur[b]
                op_ps = ps_big.tile([128, 512], F32,
                                    name=f"op{b}_{nt}_{cc}", tag="ps")
                nc.tensor.matmul(op_ps[:nsz, :], u[b][:, n0:n0 + nsz],
                                 bw[:, cc * 512:(cc + 1) * 512],
                                 start=True, stop=True)
                if cc % 2 == 0:
                    nc.scalar.copy(osb[:nsz, cc * 512:(cc + 1) * 512],
                                   op_ps[:nsz, :])
                else:
                    nc.vector.tensor_copy(
                        osb[:nsz, cc * 512:(cc + 1) * 512], op_ps[:nsz, :])
                if cc == CCH - 1:
                    nc.sync.dma_start(upd[b * N + n0:b * N + n0 + nsz, :],
                                      osb[:nsz, :])

            NTILES = (N + 127) // 128
            s2_units = [(nt, cc) for nt in range(NTILES) for cc in range(CCH)]

            # ---- emission schedule ----
            # batch 0: stage 1, routing, eviction
            ups0 = stage1_mms(0)
            routing_and_evict(0, ups0)
            # batch 0 stage 2 interleaved with batch 1 stage 1 (two k-slices
            # after each n-tile), so PE stays busy through batch-1 token
            # arrival and the output stream starts as early as possible
            ups1 = [ps_big.tile([128, 512], F32, name=f"up1_{c0}", tag="ps")
                    for (c0, cw) in S1_CHUNKS]
            for nt in range(NTILES):
                for cc in range(CCH):
                    stage2_unit(0, nt, cc)
                for k in ([2 * nt, 2 * nt + 1] if nt < KT // 2 else []):
                    for (c0, cw), up in zip(S1_CHUNKS, ups1):
                        nc.tensor.matmul(up[:, :cw], la[:, k, :],
                                         tk[(1, k)][:, c0:c0 + cw],
                                         start=(k == 0), stop=(k == KT - 1))
            routing_and_evict(1, ups1)
            # batch 1 stage 2
            for nt in range(NTILES):
                for cc in range(CCH):
                    stage2_unit(1, nt, cc)

    _split_waits(nc)
    return nc


_NC_CACHE = {}


def _get_nc():
    if "nc" not in _NC_CACHE:
        _NC_CACHE["nc"] = _build_nc()
    return _NC_CACHE["nc"]


def make_in_maps(tokens, gate_w, lora_a, lora_b, scaling):
    # replicated params, kernel-friendly layouts (pure relayout, no math)
    # lora_ah[p, k, m] = lora_a[(m//R), (m%R), k*128+p]
    lora_ah = np.ascontiguousarray(
        lora_a.reshape(ER, KT, 128).transpose(2, 1, 0))  # [128, KT, ER]
    lora_bt = np.ascontiguousarray(
        lora_b.transpose(0, 2, 1).reshape(ER, C3))       # [ER, C3]
    # gate_h[p, k, e] = gate_w[k*128+p, e]
    gate_h = np.ascontiguousarray(
        gate_w.reshape(KT, 128, E).transpose(1, 0, 2))   # [128, KT, E]
    srep = np.ascontiguousarray(
        np.repeat(scaling, R).reshape(ER, 1))            # [ER, 1]
    in_maps = []
    for c in range(NCORES):
        shard = tokens[c * BPC:(c + 1) * BPC]            # [BPC, N, D]
        # tok_h[b, p, k, n] = tokens[b, n, k*128+p], zero-padded to NP cols
        tok_h = np.zeros((BPC, 128, KT, NP), dtype=np.float32)
        tok_h[:, :, :, :N] = shard.transpose(0, 2, 1).reshape(
            BPC, KT, 128, N).transpose(0, 2, 1, 3)
        in_maps.append({
            "tok_h": tok_h, "gate_h": gate_h, "lora_ah": lora_ah,
            "lora_bt": lora_bt, "srep": srep,
        })
    return in_maps


def kernel(tokens, gate_w, lora_a, lora_b, scaling):
    tokens = np.asarray(tokens, dtype=np.float32)
    gate_w = np.asarray(gate_w, dtype=np.float32)
    lora_a = np.asarray(lora_a, dtype=np.float32)
    lora_b = np.asarray(lora_b, dtype=np.float32)
    scaling = np.asarray(scaling, dtype=np.float32)

    in_maps = make_in_maps(tokens, gate_w, lora_a, lora_b, scaling)
    res = run_bass_kernel_spmd(_get_nc(), in_maps, core_ids=list(range(NCORES)))

    weighted_update = np.empty((B, N, C3), dtype=np.float32)
    router_logits = np.empty((B, E), dtype=np.float32)
    for c in range(NCORES):
        weighted_update[c * BPC:(c + 1) * BPC] = \
            res.results[c]["upd"].reshape(BPC, N, C3)
        router_logits[c * BPC:(c + 1) * BPC] = res.results[c]["logits"]

    # [16,8]-sized routing outputs, derived from device-computed logits
    top_idx = np.argsort(-router_logits, axis=-1, kind="stable")[:, :TOP_K]
    top_idx = np.ascontiguousarray(top_idx).astype(np.int32)
    top_vals = np.take_along_axis(router_logits, top_idx, axis=1)
    ex = np.exp(top_vals - top_vals.max(axis=1, keepdims=True))
    top_w = (ex / ex.sum(axis=1, keepdims=True)).astype(np.float32)
    expert_weights = np.zeros((B, E), dtype=np.float32)
    np.put_along_axis(expert_weights, top_idx, top_w, axis=1)
    importance = expert_weights.sum(axis=0)
    load = np.zeros((E,), dtype=np.float32)
    np.add.at(load, top_idx.reshape(-1), 1.0)

    return (weighted_update, router_logits, top_idx, expert_weights,
            importance, load)
